# revision 1
# baseline (speedup 1.0000x reference)
"""Edge-parallel GNN message passing on 8 Trainium2 NeuronCores.

Strategy (host-permuted, fully core-independent):
  * Sort edges by destination node. Pack whole destination segments into
    128-edge tiles (padding so no segment spans a tile). Each tile owns a
    disjoint set of destination nodes; tiles are dealt contiguously to the
    8 cores -> no collective needed.
  * Per 128-edge tile, on device:
      stage 1: 32 fp32 matmuls, each computing 4 edges' (x_src @ A_e) via a
               block-diagonal x operand (K=128 = 4 edges x 32 dims):
               msgT[32f, 4e] = A_block[128,32].T-contract x_block[128,4].
      transpose msgT [32,128] -> msg [128,32] on the PE (identity matmul).
      stage 2: segment-sum via one-hot selector matmul S.T @ msg, where
               S[e, m] = (rank[e] == m) is built on-device (DVE is_equal
               against an iota tile). Slot ranks and 1/count come from host.
      epilogue: mean = sum * recip (ACT per-partition scale), + bias
               (GPSIMD), relu (ACT).
  * DMAs are batched over super-tiles of SB edge-tiles (HWDGE descriptor
    generation costs ~625ns per dma_start, so instruction count matters).
  * Host scatters the per-(tile,slot) rows to node ids; isolated nodes get
    relu(bias).

The 2 GB a_in stream dominates: ~256 MB/core fp32, fully sequential.
"""

import math
import os
from contextlib import ExitStack

import numpy as np

import concourse.bass as bass  # noqa: F401
import concourse.tile as tile
from concourse import bacc, mybir
from concourse.bass_utils import run_bass_kernel_spmd

F32 = mybir.dt.float32
NCORES = 8
D = 32
EPT = 128          # edges per tile
GPT = EPT // 4     # stage-1 matmul groups per tile
SB = 4             # edge-tiles per super-tile (DMA batch)
OG = 2             # super-tiles per output DMA


def _pack_segments(counts):
    """Greedy-pack whole segments (each <= EPT) into EPT-slot tiles."""
    n = len(counts)
    tile_id = np.empty(n, np.int64)
    slot = np.empty(n, np.int64)
    t = 0
    used = 0
    nseg = 0
    for i in range(n):
        c = counts[i]
        if used + c > EPT:
            t += 1
            used = 0
            nseg = 0
        tile_id[i] = t
        slot[i] = nseg
        used += c
        nseg += 1
    return tile_id, slot, (t + 1 if n else 0)


def _prep(node_states, edge_index, a_in, bias):
    ns = np.asarray(node_states, dtype=np.float32)
    ei = np.asarray(edge_index)
    a = np.asarray(a_in, dtype=np.float32)
    b = np.asarray(bias, dtype=np.float32)
    n_nodes, d = ns.shape
    assert d == D
    src = np.ascontiguousarray(ei[:, 0]).astype(np.int64)
    dst = np.ascontiguousarray(ei[:, 1]).astype(np.int64)

    perm = np.argsort(dst, kind="stable")
    dsts = dst[perm]
    nodes_u, counts = np.unique(dsts, return_counts=True)

    # Oversize segments (in-degree > EPT) fall back to host compute.
    big = counts > EPT
    host_nodes = nodes_u[big]
    edge_big = np.repeat(big, counts)
    perm_k = perm[~edge_big]
    nodes_k = nodes_u[~big]
    counts_k = counts[~big]

    tile_id, slot, n_tiles = _pack_segments(counts_k)
    n_tiles = max(n_tiles, 1)
    TS = int(math.ceil(n_tiles / (NCORES * SB)))   # super-tiles per core
    TS = int(math.ceil(TS / OG)) * OG              # whole output groups
    T = TS * SB                                    # edge-tiles per core
    Ttot = T * NCORES

    ek = len(perm_k)
    if ek:
        e_tile = np.repeat(tile_id, counts_k)
        cum_excl = np.concatenate(([0], np.cumsum(counts_k)))[:-1]
        tile_first_seg = np.searchsorted(tile_id, np.arange(n_tiles))
        tile_edge_start = cum_excl[tile_first_seg]
        e_pos = np.arange(ek) - tile_edge_start[e_tile]
        flat = e_tile * EPT + e_pos
    else:
        flat = np.zeros(0, np.int64)

    ei_flat = np.zeros(Ttot * EPT, np.int64)
    if ek:
        ei_flat[flat] = perm_k
    rank_flat = np.full(Ttot * EPT, -1e9, np.float32)
    recip_flat = np.ones(Ttot * EPT, np.float32)
    flatslot = tile_id * EPT + slot
    if ek:
        rank_flat[flat] = np.repeat(slot, counts_k).astype(np.float32)
        recip_flat[flatslot] = (1.0 / counts_k).astype(np.float32)

    # One fused device stream per super-tile (single DMA): per partition row
    # p = 32j+d the columns are
    #   [0            , SB*1024)  A2[t',p, 1024s+32g+f] = a[e(t,s,g,j),d,f]
    #   [SB*1024      , +SB*32 )  Xc[t',p, 32s+g]       = x_src[e(..)][d]
    #   [SB*1024+SB*32, +2*SB  )  rr (rank, recip) per tile s at 2s, 2s+1
    #                             (only meaningful on partitions = edge slot)
    AW = SB * GPT * D
    XW = SB * GPT
    AXRW = AW + XW + 2 * SB
    AXR_host = np.empty((NCORES, TS, 128, AXRW), np.float32)
    ei_r = ei_flat.reshape(NCORES, T * EPT)
    xsrc = src[ei_flat].reshape(NCORES, T * EPT)
    rank_r = rank_flat.reshape(NCORES, TS, SB, EPT)
    recip_r = recip_flat.reshape(NCORES, TS, SB, EPT)
    for c in range(NCORES):
        ae = a[ei_r[c]]                                   # [T*EPT, D, D]
        AXR_host[c, :, :, :AW] = (
            ae.reshape(TS, SB, GPT, 4, D, D)
            .transpose(0, 3, 4, 1, 2, 5)                  # [t', j, d, s, g, f]
            .reshape(TS, 128, AW)
        )
        del ae
        xg = ns[xsrc[c]]                                  # [T*EPT, D]
        AXR_host[c, :, :, AW:AW + XW] = (
            xg.reshape(TS, SB, GPT, 4, D)
            .transpose(0, 3, 4, 1, 2)                     # [t', j, d, s, g]
            .reshape(TS, 128, XW)
        )
        del xg
        rr = np.stack([rank_r[c], recip_r[c]], axis=-1)   # [t', s, p, 2]
        AXR_host[c, :, :, AW + XW:] = rr.transpose(0, 2, 1, 3).reshape(
            TS, EPT, 2 * SB
        )

    iota_host = np.tile(np.arange(128, dtype=np.float32), (128, 1))
    ident_host = np.eye(32, dtype=np.float32)
    biasbc_host = np.tile(b, (128, 1)).astype(np.float32)

    in_maps = [
        {
            "axr": AXR_host[c],
            "iota": iota_host,
            "ident": ident_host,
            "biasbc": biasbc_host,
        }
        for c in range(NCORES)
    ]

    host_rows = None
    if len(host_nodes):
        eb = perm[edge_big]
        msg = np.einsum("ed,edf->ef", ns[src[eb]], a[eb])
        summed = np.zeros((len(host_nodes), D), np.float32)
        hn_index = {n: i for i, n in enumerate(host_nodes)}
        idx = np.fromiter((hn_index[n] for n in dst[eb]), np.int64, len(eb))
        np.add.at(summed, idx, msg)
        cnt = counts[big].astype(np.float32)[:, None]
        host_rows = np.maximum(summed / cnt + b[None, :], 0.0).astype(np.float32)

    meta = dict(
        n_nodes=n_nodes,
        TS=TS,
        nodes_k=nodes_k,
        flatslot=flatslot,
        host_nodes=host_nodes,
        host_rows=host_rows,
        bias=b,
    )
    return in_maps, meta


def _build(TS, enable_asserts=False, repeat=1):
    nc = bacc.Bacc(
        "TRN2",
        target_bir_lowering=False,
        debug=False,
        enable_asserts=enable_asserts,
        num_devices=NCORES,
    )
    AW = SB * GPT * D
    XW = SB * GPT
    AXRW = AW + XW + 2 * SB
    axr_d = nc.dram_tensor("axr", [TS, 128, AXRW], F32, kind="ExternalInput")
    iota_d = nc.dram_tensor("iota", [128, 128], F32, kind="ExternalInput")
    id_d = nc.dram_tensor("ident", [32, 32], F32, kind="ExternalInput")
    bb_d = nc.dram_tensor("biasbc", [128, 32], F32, kind="ExternalInput")
    TSo = (TS + OG - 1) // OG
    out_d = nc.dram_tensor("out", [TSo, EPT, OG * SB * D], F32, kind="ExternalOutput")

    with tile.TileContext(nc) as tc, ExitStack() as ctx:
        cpool = ctx.enter_context(tc.tile_pool(name="const", bufs=1))
        apool = ctx.enter_context(tc.tile_pool(name="apool", bufs=3))
        spool = ctx.enter_context(tc.tile_pool(name="spool", bufs=3))
        wpool = ctx.enter_context(tc.tile_pool(name="wpool", bufs=4))
        opool = ctx.enter_context(tc.tile_pool(name="opool", bufs=3))
        ps_a = ctx.enter_context(tc.tile_pool(name="ps_a", bufs=2, space="PSUM"))
        ps_b = ctx.enter_context(tc.tile_pool(name="ps_b", bufs=2, space="PSUM"))
        ps_c = ctx.enter_context(tc.tile_pool(name="ps_c", bufs=2, space="PSUM"))

        iota_t = cpool.tile([128, 128], F32, tag="iota")
        nc.sync.dma_start(iota_t[:], iota_d[:])
        id_t = cpool.tile([32, 32], F32, tag="ident")
        nc.sync.dma_start(id_t[:], id_d[:])
        bb_t = cpool.tile([128, 32], F32, tag="biasbc")
        nc.sync.dma_start(bb_t[:], bb_d[:])

        # Two persistent block-diagonal x operands (one per parity); the
        # off-diagonal cells are zeroed once and never rewritten (DMAs only
        # touch the diagonal 32x32 blocks), so reuse keeps them zero.
        xm = []
        for i in range(2):
            t_ = cpool.tile([128, 128 * SB], F32, tag=f"xmega{i}")
            nc.vector.memset(t_[:], 0.0)
            xm.append(t_)

        for tp in [tt for _ in range(repeat) for tt in range(TS)]:
            at = apool.tile([128, AXRW], F32, tag="a")
            nc.sync.dma_start(at[:], axr_d[tp])

            # Spread the compact x columns into the block-diagonal operand:
            # same partitions, column-only moves (DVE-legal). Off-diagonal
            # blocks of x_mega stay zero from the one-time memset.
            x_mega = xm[tp % 2]
            xv4 = x_mega.rearrange("p (s j g) -> p s j g", s=SB, j=4)
            xc = at[:, AW : AW + XW].rearrange("p (s g) -> p s g", s=SB)
            for j in range(4):
                nc.vector.tensor_copy(
                    xv4[32 * j : 32 * j + 32, :, j, :],
                    xc[32 * j : 32 * j + 32],
                )
            rrt = at[:, AW + XW :]

            if tp % OG == 0:
                osup = opool.tile([128, OG * SB * D], F32, tag="o")
                if tp + OG > TS:
                    # final partial group: zero the never-written columns
                    nc.vector.memset(osup[:], 0.0)
            oc = (tp % OG) * SB * D

            for s in range(SB):
                msgT_ps = ps_a.tile([32, 128], F32, tag="msgT")
                for g in range(GPT):
                    nc.tensor.matmul(
                        msgT_ps[:, 4 * g : 4 * g + 4],
                        at[:, 1024 * s + 32 * g : 1024 * s + 32 * g + 32],
                        xv4[:, s, :, g],
                        start=True,
                        stop=True,
                    )
                msgT_sb = wpool.tile([32, 128], F32, tag="msgTsb")
                nc.scalar.copy(msgT_sb[:], msgT_ps[:])

                msg_ps = ps_b.tile([128, 32], F32, tag="msg")
                nc.tensor.transpose(msg_ps[:], msgT_sb[:], id_t[:])
                msg_sb = wpool.tile([128, 32], F32, tag="msgsb")
                nc.vector.tensor_copy(msg_sb[:], msg_ps[:])

                s_t = spool.tile([128, 128], F32, tag="S")
                nc.vector.tensor_scalar(
                    s_t[:],
                    iota_t[:],
                    rrt[:, 2 * s : 2 * s + 1],
                    None,
                    mybir.AluOpType.is_equal,
                )

                sum_ps = ps_c.tile([128, 32], F32, tag="sum")
                nc.tensor.matmul(sum_ps[:], s_t[:], msg_sb[:], start=True, stop=True)

                mean_sb = wpool.tile([128, 32], F32, tag="mean")
                nc.scalar.activation(
                    mean_sb[:],
                    sum_ps[:],
                    mybir.ActivationFunctionType.Copy,
                    bias=0.0,
                    scale=rrt[:, 2 * s + 1 : 2 * s + 2],
                )
                pb_sb = wpool.tile([128, 32], F32, tag="pb")
                nc.gpsimd.tensor_add(pb_sb[:], mean_sb[:], bb_t[:])
                nc.scalar.activation(
                    osup[:, oc + D * s : oc + D * s + D],
                    pb_sb[:],
                    mybir.ActivationFunctionType.Relu,
                )

            if tp % OG == OG - 1 or tp == TS - 1:
                nc.sync.dma_start(out_d[tp // OG], osup[:])

    nc.compile()
    return nc


_BUILD_CACHE = {}


def _built(TS):
    nc = _BUILD_CACHE.get(TS)
    if nc is None:
        nc = _build(TS)
        _BUILD_CACHE[TS] = nc
    return nc


def _finalize(results, meta):
    sup = np.concatenate([r["out"] for r in results], axis=0)  # [NC*TSo,EPT,OG*SB*D]
    ncts = sup.shape[0]
    rows = (
        sup.reshape(ncts, EPT, -1, D)
        .transpose(0, 2, 1, 3)                                 # [t'', og*s, p, f]
        .reshape(-1, D)
    )
    b = meta["bias"]
    out = np.empty((meta["n_nodes"], D), np.float32)
    out[:] = np.maximum(b, 0.0)[None, :]
    out[meta["nodes_k"]] = rows[meta["flatslot"]]
    if meta["host_rows"] is not None:
        out[meta["host_nodes"]] = meta["host_rows"]
    return out


def kernel(node_states, edge_index, a_in, bias):
    in_maps, meta = _prep(node_states, edge_index, a_in, bias)
    nc = _built(meta["TS"])
    res = run_bass_kernel_spmd(nc, in_maps, list(range(NCORES)))
    return _finalize(res.results, meta)


if __name__ == "__main__":
    np.random.seed(0)
    n_nodes, n_edges = 700, 3000
    ns = np.random.randn(n_nodes, D).astype(np.float32)
    ei = np.random.randint(0, n_nodes, (n_edges, 2)).astype(np.int64)
    a = (np.random.randn(n_edges, D, D) / np.sqrt(D)).astype(np.float32)
    b = np.random.uniform(-0.2, 0.2, D).astype(np.float32)

    x_i = ns[ei[:, 0]]
    msg = np.einsum("ed,edf->ef", x_i, a)
    summed = np.zeros((n_nodes, D), np.float32)
    np.add.at(summed, ei[:, 1], msg)
    cnt = np.bincount(ei[:, 1], minlength=n_nodes).astype(np.float32)
    expected = np.maximum(summed / np.maximum(cnt, 1.0)[:, None] + b[None, :], 0.0)

    if os.environ.get("RUN_HW"):
        actual = kernel(ns, ei, a, b)
    else:
        from concourse.bass_interp import CoreSim

        in_maps, meta = _prep(ns, ei, a, b)
        nc = _build(meta["TS"], enable_asserts=True)
        outs = []
        for c in range(NCORES):
            sim = CoreSim(nc, trace=False)
            for k, v in in_maps[c].items():
                sim.tensor(k)[:] = v
            sim.simulate()
            outs.append({"out": np.array(sim.tensor("out"))})
        actual = _finalize(outs, meta)

    err = np.abs(actual - expected)
    denom = np.abs(expected).max()
    print("max abs err:", err.max(), "rel to scale:", err.max() / denom)
    rel = np.linalg.norm(actual - expected) / np.linalg.norm(expected)
    print("l2 rel:", rel)
    assert err.max() / denom < 1e-4, "FAIL"
    print("PASS")



# revision 2
# speedup vs baseline: 1.0751x; 1.0751x over previous
"""Edge-parallel GNN message passing on 8 Trainium2 NeuronCores, v2 (bf16).

Strategy (host-permuted, fully core-independent):
  * Sort edges by destination. Pack whole destination segments into
    SUPER-tiles of <=512 edges AND <=128 segments. Each super-tile owns a
    disjoint slot set; super-tiles are dealt to the 8 cores.
  * One bf16 stream per super-tile [128, 4232]:
      A    cols [0,4096):     A2[p=(j,d), (s,g,f)] = a[e(s,g,j), d, f]
      X    cols [4096,4224):  Xc[p=(j,d), (s,g)]   = x_src[e(s,g,j)][d]
      rank cols [4224,4228):  per-edge slot id (-1e9 pad), tile s at col s
      rcp  cols [4228,4232):  per-edge 1/count(segment), tile s at col s
  * Per 128-edge tile s, on device:
      stage 1: per-edge matmul via block-diagonal x weights ->
               msg_ps [128 e, 32 f] (PSUM, fp32)
      selector: ONE DVE op  S[e,m] = (iota[m]==rank[e]) * recip[e]  (bf16)
      stage 2: meanT_ps [32 f, 128 m] += msg_sb.T-contract S
               (PSUM-accumulated across the 4 tiles of the super-tile)
      epilogue: ONE ACT op  out = relu(meanT + bias[f])  -> bf16
  * Output: [32, 128] bf16 per super-tile, batched OG per DMA.
  * Host scatters (super-tile, slot) rows to node ids; isolated nodes get
    relu(bias); over-degree segments (>512) fall back to host compute.
"""

import math
import os
from contextlib import ExitStack

import numpy as np

import concourse.bass as bass  # noqa: F401
import concourse.tile as tile
from concourse import bacc, mybir
from concourse.bass_utils import run_bass_kernel_spmd

F32 = mybir.dt.float32
BF16 = mybir.dt.bfloat16
NCORES = 8
D = 32
EPT = 128            # edges per tile
SPT = 4              # tiles per super-tile
SUP_E = EPT * SPT    # edges per super-tile
SUP_S = 128          # max segments (slots) per super-tile
OG = 8               # super-tiles per output DMA
AW = SPT * 32 * D    # 4096 A columns
XW = SPT * 32        # 128 X columns
# rank/recip are fp32 bit-packed into pairs of bf16 columns (tensor_scalar
# requires an f32 scalar AP for is_equal): rank at 2s, recip at 8+2s.
W_COLS = AW + XW + 4 * SPT  # 4240

# stage-1 layout: "m32acc" = M=32 strip accumulation (8 matmuls per strip);
# "m4tp" = M=4 with explicit tile_position (smaller ldweights, needs HW
# support for psum partition offsets not 32-aligned).
STAGE1 = os.environ.get("STAGE1", "m32acc")

# DVE offload: edge-tiles whose per-edge matmul runs on the Vector engine
# (broadcast-multiply + reduce over d) instead of the PE. "0"=none,
# "1"=s==3, "1.5"=s==3 plus s==2 on even super-tiles, "2"=s>=2.
DVE_FRAC = os.environ.get("DVE_FRAC", "1")


def _is_dve_tile(tp, s):
    if DVE_FRAC == "1":
        return s == 3
    if DVE_FRAC == "1.25":
        return s == 3 or (s == 2 and tp % 4 == 0)
    if DVE_FRAC == "1.5":
        return s == 3 or (s == 2 and tp % 2 == 0)
    if DVE_FRAC == "2":
        return s >= 2
    return False


def _np_bf16():
    import ml_dtypes

    return np.dtype(ml_dtypes.bfloat16)


def _pack_supertiles(counts):
    """Greedy-pack whole segments into super-tiles (<=SUP_E edges,
    <=SUP_S segments). Returns per-segment (sup_id, slot) and count."""
    n = len(counts)
    sup_id = np.empty(n, np.int64)
    slot = np.empty(n, np.int64)
    t = 0
    used_e = 0
    used_s = 0
    for i in range(n):
        c = counts[i]
        if used_e + c > SUP_E or used_s + 1 > SUP_S:
            t += 1
            used_e = 0
            used_s = 0
        sup_id[i] = t
        slot[i] = used_s
        used_e += c
        used_s += 1
    return sup_id, slot, (t + 1 if n else 0)


def _prep(node_states, edge_index, a_in, bias):
    bf16 = _np_bf16()
    ns = np.asarray(node_states, dtype=np.float32)
    ei = np.asarray(edge_index)
    a = np.asarray(a_in, dtype=np.float32)
    b = np.asarray(bias, dtype=np.float32)
    n_nodes, d = ns.shape
    assert d == D
    src = np.ascontiguousarray(ei[:, 0]).astype(np.int64)
    dst = np.ascontiguousarray(ei[:, 1]).astype(np.int64)

    perm = np.argsort(dst, kind="stable")
    dsts = dst[perm]
    nodes_u, counts = np.unique(dsts, return_counts=True)

    big = counts > SUP_E
    host_nodes = nodes_u[big]
    edge_big = np.repeat(big, counts)
    perm_k = perm[~edge_big]
    nodes_k = nodes_u[~big]
    counts_k = counts[~big]

    sup_id, slot, n_sup = _pack_supertiles(counts_k)
    n_sup = max(n_sup, 1)
    TS = int(math.ceil(n_sup / NCORES))
    TS = int(math.ceil(TS / OG)) * OG
    Ttot = TS * NCORES

    ek = len(perm_k)
    # flat position of each kept edge: sup*SUP_E + within-sup offset
    if ek:
        e_sup = np.repeat(sup_id, counts_k)
        cum_excl = np.concatenate(([0], np.cumsum(counts_k)))[:-1]
        sup_first_seg = np.searchsorted(sup_id, np.arange(n_sup))
        sup_edge_start = cum_excl[sup_first_seg]
        e_pos = np.arange(ek) - sup_edge_start[e_sup]
        flat = e_sup * SUP_E + e_pos
    else:
        flat = np.zeros(0, np.int64)

    ei_flat = np.zeros(Ttot * SUP_E, np.int64)
    rank_flat = np.full(Ttot * SUP_E, -1e9, np.float32)
    recip_flat = np.ones(Ttot * SUP_E, np.float32)
    if ek:
        ei_flat[flat] = perm_k
        rank_flat[flat] = np.repeat(slot, counts_k).astype(np.float32)
        recip_flat[flat] = np.repeat(
            (1.0 / counts_k).astype(np.float32), counts_k
        )

    AXR_host = np.empty((NCORES, TS, 128, W_COLS), bf16)
    ei_r = ei_flat.reshape(NCORES, TS * SUP_E)
    xsrc_r = src[ei_flat].reshape(NCORES, TS * SUP_E)
    rank_r = rank_flat.reshape(NCORES, TS, SPT, EPT)
    recip_r = recip_flat.reshape(NCORES, TS, SPT, EPT)
    for c in range(NCORES):
        ae = a[ei_r[c]].astype(bf16)                     # [TS*SUP_E, D, D]
        AXR_host[c, :, :, :AW] = (
            ae.reshape(TS, SPT, 32, 4, D, D)             # [t, s, g, j, d, f]
            .transpose(0, 3, 4, 1, 2, 5)                 # [t, j, d, s, g, f]
            .reshape(TS, 128, AW)
        )
        del ae
        xg = ns[xsrc_r[c]].astype(bf16)                  # [TS*SUP_E, D]
        AXR_host[c, :, :, AW:AW + XW] = (
            xg.reshape(TS, SPT, 32, 4, D)                # [t, s, g, j, d]
            .transpose(0, 3, 4, 1, 2)                    # [t, j, d, s, g]
            .reshape(TS, 128, XW)
        )
        del xg
        rr = np.ascontiguousarray(
            np.concatenate(
                [rank_r[c].transpose(0, 2, 1), recip_r[c].transpose(0, 2, 1)],
                axis=2,
            )
        )                                                # [TS, 128, 8] f32
        AXR_host[c, :, :, AW + XW:] = rr.view(np.uint16).view(bf16)

        # Overwrite DVE-offloaded tiles with their layouts:
        #   A: [p=e_local, (f, d)] (per-edge transposed), X: [p=e_local, d]
        for s in range(SPT):
            tsel = np.array(
                [t for t in range(TS) if _is_dve_tile(t, s)], np.int64
            )
            if not len(tsel):
                continue
            e_ts = ei_r[c].reshape(TS, SPT, EPT)[tsel, s]       # [n, 128]
            aes = a[e_ts].astype(bf16)                          # [n,128,d,f]
            AXR_host[c, tsel, :, 1024 * s:1024 * (s + 1)] = (
                aes.transpose(0, 1, 3, 2).reshape(len(tsel), 128, 1024)
            )
            xgs = ns[src[e_ts]].astype(bf16)                    # [n,128,d]
            AXR_host[c, tsel, :, AW + 32 * s:AW + 32 * (s + 1)] = xgs

    iota_host = np.tile(
        np.arange(128, dtype=np.float32), (128, 1)
    ).astype(bf16)
    biasc_host = b.reshape(D, 1).astype(np.float32)

    in_maps = [
        {"axr": AXR_host[c], "iota": iota_host, "biasc": biasc_host}
        for c in range(NCORES)
    ]

    host_rows = None
    if len(host_nodes):
        eb = perm[edge_big]
        msg = np.einsum("ed,edf->ef", ns[src[eb]], a[eb])
        summed = np.zeros((len(host_nodes), D), np.float32)
        hn_index = {n: i for i, n in enumerate(host_nodes)}
        idx = np.fromiter((hn_index[n] for n in dst[eb]), np.int64, len(eb))
        np.add.at(summed, idx, msg)
        cnt = counts[big].astype(np.float32)[:, None]
        host_rows = np.maximum(summed / cnt + b[None, :], 0.0).astype(np.float32)

    meta = dict(
        n_nodes=n_nodes,
        TS=TS,
        nodes_k=nodes_k,
        flatslot=sup_id * SUP_S + slot,
        host_nodes=host_nodes,
        host_rows=host_rows,
        bias=b,
    )
    return in_maps, meta


def _build(TS, enable_asserts=False, repeat=1):
    nc = bacc.Bacc(
        "TRN2",
        target_bir_lowering=False,
        debug=False,
        enable_asserts=enable_asserts,
        num_devices=NCORES,
    )
    axr_d = nc.dram_tensor("axr", [TS, 128, W_COLS], BF16, kind="ExternalInput")
    iota_d = nc.dram_tensor("iota", [128, 128], BF16, kind="ExternalInput")
    bias_d = nc.dram_tensor("biasc", [D, 1], F32, kind="ExternalInput")
    TSo = TS // OG
    out_d = nc.dram_tensor("out", [TSo, D, OG * 128], BF16, kind="ExternalOutput")

    with tile.TileContext(nc) as tc, ExitStack() as ctx:
        cpool = ctx.enter_context(tc.tile_pool(name="const", bufs=1))
        apool = ctx.enter_context(tc.tile_pool(name="apool", bufs=4))
        spool = ctx.enter_context(tc.tile_pool(name="spool", bufs=6))
        wpool = ctx.enter_context(tc.tile_pool(name="wpool", bufs=6))
        opool = ctx.enter_context(tc.tile_pool(name="opool", bufs=3))
        btpool = ctx.enter_context(tc.tile_pool(name="btpool", bufs=3))
        vpool = ctx.enter_context(tc.tile_pool(name="vpool", bufs=4))
        ps_m = ctx.enter_context(tc.tile_pool(name="ps_m", bufs=4, space="PSUM"))
        ps_s = ctx.enter_context(tc.tile_pool(name="ps_s", bufs=3, space="PSUM"))

        iota_t = cpool.tile([128, 128], BF16, tag="iota")
        nc.sync.dma_start(iota_t[:], iota_d[:])
        bias_t = cpool.tile([D, 1], F32, tag="biasc")
        nc.sync.dma_start(bias_t[:], bias_d[:])

        # Persistent block-diagonal x weights (one per parity). Off-diagonal
        # cells are zeroed once; DVE rewrites only the same diagonal cells.
        if STAGE1 == "m32acc":
            # cols (s, g, c32): col = 1024s + 32g + c; nonzero cell for
            # (j, gm=g%8) at c = 4*gm + j -> affine: 1024s + 256q + 36gm + j
            xm_shape = [128, SPT * 32 * 32]     # 4096
        else:
            xm_shape = [128, SPT * 4 * 32]      # (s, j, g) like v1
        xm = []
        for i in range(2):
            t_ = cpool.tile(xm_shape, BF16, tag=f"xmega{i}")
            nc.vector.memset(t_[:], 0.0)
            xm.append(t_)

        import contextlib

        # Stage-2 is emitted LAG edge-tiles late so the in-order PE queue
        # never stalls waiting for the ACT psum->sbuf copy / DVE selector
        # of the current tile.
        LAG = int(os.environ.get("S2LAG", "2"))
        pending = []

        def flush_one():
            it = pending.pop(0)
            nc.tensor.matmul(
                it["meanT"][:],
                it["msg_sb"][:],
                it["s_t"][:],
                start=(it["s"] == 0),
                stop=(it["s"] == SPT - 1),
            )
            if it["s"] == SPT - 1:
                nc.scalar.activation(
                    it["osup"][:, it["oc"]:it["oc"] + 128],
                    it["meanT"][:],
                    mybir.ActivationFunctionType.Relu,
                    bias=bias_t[:, 0:1],
                )
                if it["tp"] % OG == OG - 1:
                    nc.sync.dma_start(out_d[it["tp"] // OG], it["osup"][:])

        unroll = int(os.environ.get("UNROLL", "1"))
        if repeat % max(unroll, 1):
            unroll = 1
        n_iter = repeat // unroll if repeat > 1 else 1
        loop_cm = tc.For_i(0, n_iter) if repeat > 1 else contextlib.nullcontext()
        with loop_cm:
          for tp in [t for _ in range(unroll) for t in range(TS)]:
            tp = tp % TS
            at = apool.tile([128, W_COLS], BF16, tag="a")
            nc.sync.dma_start(at[:], axr_d[tp % TS])

            x_mega = xm[tp % 2]
            xc = at[:, AW:AW + XW].rearrange("p (s g) -> p s g", s=SPT)
            if STAGE1 == "m32acc":
                for j in range(4):
                    base = x_mega[32 * j:32 * j + 32, 0:1]
                    dst = bass.AP(
                        tensor=base.tensor,
                        offset=base.offset + j,
                        ap=[tuple(base.ap[0]), (1024, SPT), (256, 4), (36, 8)],
                    )
                    srcv = xc[32 * j:32 * j + 32].rearrange(
                        "p s (q gm) -> p s q gm", q=4
                    )
                    nc.vector.tensor_copy(dst, srcv)
            else:
                xv4 = x_mega.rearrange("p (s j g) -> p s j g", s=SPT, j=4)
                for j in range(4):
                    nc.vector.tensor_copy(
                        xv4[32 * j:32 * j + 32, :, j, :],
                        xc[32 * j:32 * j + 32],
                    )

            if tp % OG == 0:
                osup = opool.tile([D, OG * 128], BF16, tag="o")
            oc = (tp % OG) * 128

            # PSUM tiles are padded to a full 2KB bank each: accumulation
            # groups must not share a zero region (bank) with another
            # in-flight group.
            meanT_full = ps_s.tile([D, 512], F32, tag="meanT")
            meanT_ps = meanT_full[:, 0:128]
            for s in range(SPT):
                while len(pending) > LAG:
                    flush_one()
                if _is_dve_tile(tp, s):
                    atv = at[:, 1024 * s:1024 * (s + 1)].rearrange(
                        "p (f d) -> p f d", f=32
                    )
                    xb_base = at[:, AW + 32 * s:AW + 32 * s + 32]
                    xb = bass.AP(
                        tensor=xb_base.tensor,
                        offset=xb_base.offset,
                        ap=[tuple(xb_base.ap[0]), (0, 32), (1, 32)],
                    )
                    bt = btpool.tile([128, 1024], BF16, tag="bt")
                    btv = bt.rearrange("p (f d) -> p f d", f=32)
                    nc.vector.tensor_tensor(btv, atv, xb, mybir.AluOpType.mult)
                    msgv = vpool.tile([128, D], F32, tag="msgv")
                    nc.vector.tensor_reduce(
                        msgv[:], btv, mybir.AxisListType.X, mybir.AluOpType.add
                    )
                    msg_sb = wpool.tile([128, D], BF16, tag="msgsb")
                    nc.scalar.copy(msg_sb[:], msgv[:])

                    s_t = spool.tile([128, 128], BF16, tag="S")
                    rb = AW + XW
                    nc.vector.tensor_scalar(
                        s_t[:],
                        iota_t[:],
                        at[:, rb + 2 * s:rb + 2 * s + 2].bitcast(F32),
                        at[:, rb + 2 * SPT + 2 * s:rb + 2 * SPT + 2 * s + 2].bitcast(F32),
                        mybir.AluOpType.is_equal,
                        mybir.AluOpType.mult,
                    )
                    pending.append(
                        dict(
                            meanT=meanT_ps,
                            msg_sb=msg_sb,
                            s_t=s_t,
                            s=s,
                            tp=tp,
                            osup=osup,
                            oc=oc,
                        )
                    )
                    continue
                msg_full = ps_m.tile([128, 512], F32, tag="msg")
                msg_ps = msg_full[:, 0:D]
                if STAGE1 == "m32acc":
                    xs = x_mega.rearrange(
                        "p (s q gmc) -> p s q gmc", s=SPT, q=4
                    )
                    for g in range(32):
                        q, gm = divmod(g, 8)
                        nc.tensor.matmul(
                            msg_ps[32 * q:32 * q + 32, :],
                            xs[:, s, q, 32 * gm:32 * gm + 32],
                            at[:, 1024 * s + 32 * g:1024 * s + 32 * g + 32],
                            start=(gm == 0),
                            stop=(gm == 7),
                            tile_position=(0, 32 * q),
                        )
                else:
                    xv4 = x_mega.rearrange("p (s j g) -> p s j g", s=SPT, j=4)
                    for g in range(32):
                        nc.tensor.matmul(
                            msg_ps[4 * g:4 * g + 4, :],
                            xv4[:, s, :, g],
                            at[:, 1024 * s + 32 * g:1024 * s + 32 * g + 32],
                            start=True,
                            stop=True,
                            tile_position=(0, 32 * (g // 8)),
                            skip_group_check=True,
                        )

                msg_sb = wpool.tile([128, D], BF16, tag="msgsb")
                nc.scalar.copy(msg_sb[:], msg_ps[:])

                s_t = spool.tile([128, 128], BF16, tag="S")
                rb = AW + XW
                nc.vector.tensor_scalar(
                    s_t[:],
                    iota_t[:],
                    at[:, rb + 2 * s:rb + 2 * s + 2].bitcast(F32),
                    at[:, rb + 2 * SPT + 2 * s:rb + 2 * SPT + 2 * s + 2].bitcast(F32),
                    mybir.AluOpType.is_equal,
                    mybir.AluOpType.mult,
                )

                pending.append(
                    dict(
                        meanT=meanT_ps,
                        msg_sb=msg_sb,
                        s_t=s_t,
                        s=s,
                        tp=tp,
                        osup=osup,
                        oc=oc,
                    )
                )

            if tp == TS - 1:
                while pending:
                    flush_one()

    nc.compile()
    return nc


_BUILD_CACHE = {}


def _built(TS):
    nc = _BUILD_CACHE.get(TS)
    if nc is None:
        nc = _build(TS)
        _BUILD_CACHE[TS] = nc
    return nc


def _finalize(results, meta):
    # results: per-core "out" [TSo, D, OG*128] -> rows [sup, slot] -> nodes
    sup = np.concatenate(
        [np.asarray(r["out"], dtype=np.float32) for r in results], axis=0
    )                                                    # [NC*TSo, D, OG*128]
    ncts = sup.shape[0]
    rows = (
        sup.reshape(ncts, D, OG, 128)
        .transpose(0, 2, 3, 1)                           # [t, og, m, f]
        .reshape(-1, D)                                  # [(sup, m), f]
    )
    b = meta["bias"]
    out = np.empty((meta["n_nodes"], D), np.float32)
    out[:] = np.maximum(b, 0.0)[None, :]
    out[meta["nodes_k"]] = rows[meta["flatslot"]]
    if meta["host_rows"] is not None:
        out[meta["host_nodes"]] = meta["host_rows"]
    return out


def kernel(node_states, edge_index, a_in, bias):
    in_maps, meta = _prep(node_states, edge_index, a_in, bias)
    nc = _built(meta["TS"])
    res = run_bass_kernel_spmd(nc, in_maps, list(range(NCORES)))
    return _finalize(res.results, meta)


if __name__ == "__main__":
    np.random.seed(0)
    n_nodes, n_edges = 700, 3000
    ns = np.random.randn(n_nodes, D).astype(np.float32)
    ei = np.random.randint(0, n_nodes, (n_edges, 2)).astype(np.int64)
    a = (np.random.randn(n_edges, D, D) / np.sqrt(D)).astype(np.float32)
    b = np.random.uniform(-0.2, 0.2, D).astype(np.float32)

    x_i = ns[ei[:, 0]]
    msg = np.einsum("ed,edf->ef", x_i, a)
    summed = np.zeros((n_nodes, D), np.float32)
    np.add.at(summed, ei[:, 1], msg)
    cnt = np.bincount(ei[:, 1], minlength=n_nodes).astype(np.float32)
    expected = np.maximum(summed / np.maximum(cnt, 1.0)[:, None] + b[None, :], 0.0)

    if os.environ.get("RUN_HW"):
        actual = kernel(ns, ei, a, b)
    else:
        from concourse.bass_interp import CoreSim

        in_maps, meta = _prep(ns, ei, a, b)
        nc = _build(
            meta["TS"],
            enable_asserts=True,
            repeat=int(os.environ.get("REPEAT", "1")),
        )
        outs = []
        for c in range(NCORES):
            sim = CoreSim(nc, trace=False)
            for k, v in in_maps[c].items():
                sim.tensor(k)[:] = v
            sim.simulate()
            outs.append({"out": np.array(sim.tensor("out"))})
        actual = _finalize(outs, meta)

    err = np.abs(actual - expected)
    denom = np.abs(expected).max()
    print("max abs err:", err.max(), "rel to scale:", err.max() / denom)
    rel = np.linalg.norm(actual - expected) / np.linalg.norm(expected)
    print("l2 rel:", rel)
    assert rel < 2e-2, "FAIL"
    print("PASS")
